# revision 6
# baseline (speedup 1.0000x reference)
"""CosineWeights kernel for Trainium2 (Bass/Tile), SPMD over 8 NeuronCores.

Math (per batch i, head h, memory row j):
    mask2   = mask*mask                                  [H,K]
    proj    = sum_k (mask2*keys)[h,k] * mem[j,k]         [H,J]
    msq     = sum_k mask2[h,k] * mem[j,k]^2              [H,J]
    kn2     = sum_k (mask*keys)^2                        [H]
    sharp   = softplus(str)[h] * proj / sqrt(kn2*msq)    (EPS folded away; norm ~40 >> 1e-6)
    out     = softmax_j(sharp)

Sharding: data-parallel over batch dim (32 batches -> 8 cores x 4), no
cross-core communication.

Layout strategy per core:
  - memory arrives [j, k] (k contiguous). PE-transposes 128x128 blocks into
    PSUM [k, j]; DVE copies PSUM->SBUF (memT), ACT squares PSUM->SBUF (memT2).
  - proj/msq matmuls use [K=128, 32] zero-padded stationary tiles (content
    pre-scaled by softplus(str) resp. kn2) placed at column offset 8*(t%4),
    with tile_position col-group g=t//4, accumulating into one dense
    [128, 512] PSUM tile per batch: partition p = 8*t + h, free = j%512.
  - Epilogue per batch runs on the dense [128,512] tiles:
    s = exp(-0.5*ln(msq')) ; sharp = proj'*s ; exp with fused row-sum;
    cross-partition fold/broadcast via tiny onehot matmuls on the PE;
    softmax has no max-subtraction (|sharp| <= ~6 -> exp is safe in fp32).
"""

import os

import numpy as np

B, H, J, K = 32, 8, 8192, 128
N_CORES = 8
B_LOC = B // N_CORES  # 4

MEGA = 1024            # j elements per mega-tile
NBLK = MEGA // 128     # 128x128 transpose blocks per mega-tile
NT = J // MEGA         # mega-tiles per batch
NQ = MEGA // 512       # 512-wide matmul chunks per mega-tile
T_PER_I = J // 512     # 16 (512-)tiles per batch -> packed 8*t+h on 128 partitions

_NC = None
LAST_RESULTS = None
LAST_EXEC_TIME_NS = None


def _kernel_body(ctx, tc, out_d, mem_d, keys_d, str_d, mask_d):
    import concourse.bass as bass
    from concourse import masks, mybir

    nc = tc.nc
    f32 = mybir.dt.float32
    AF = mybir.ActivationFunctionType

    const_pool = ctx.enter_context(tc.tile_pool(name="const", bufs=1))
    prep_pool = ctx.enter_context(tc.tile_pool(name="prep", bufs=1))
    nat_pool = ctx.enter_context(tc.tile_pool(name="nat", bufs=3))
    memT_pool = ctx.enter_context(tc.tile_pool(name="memT", bufs=3))
    memT2_pool = ctx.enter_context(tc.tile_pool(name="memT2", bufs=3))
    epi_pool = ctx.enter_context(tc.tile_pool(name="epi", bufs=2))
    small_pool = ctx.enter_context(tc.tile_pool(name="small", bufs=2))
    psumT_pool = ctx.enter_context(
        tc.tile_pool(name="psumT", bufs=2, space=bass.MemorySpace.PSUM)
    )
    proj_pool = ctx.enter_context(
        tc.tile_pool(name="projps", bufs=1, space=bass.MemorySpace.PSUM)
    )
    msq_pool = ctx.enter_context(
        tc.tile_pool(name="msqps", bufs=1, space=bass.MemorySpace.PSUM)
    )
    tiny_pool = ctx.enter_context(
        tc.tile_pool(name="tinyps", bufs=2, space=bass.MemorySpace.PSUM)
    )

    identity = const_pool.tile([128, 128], f32)
    masks.make_identity(nc, identity[:])

    # ---- prep: per-(i,h) scalars and stationary matrices --------------------
    IH = B_LOC * H  # 32

    keys_sb = prep_pool.tile([IH, K], f32)
    nc.sync.dma_start(keys_sb[:], keys_d.rearrange("i h k -> (i h) k"))
    mask_sb = prep_pool.tile([IH, K], f32)
    nc.sync.dma_start(mask_sb[:], mask_d.rearrange("i h k -> (i h) k"))
    str_sb = prep_pool.tile([IH, 1], f32)
    nc.sync.dma_start(str_sb[:], str_d.rearrange("i h one -> (i h) one"))

    mask2 = prep_pool.tile([IH, K], f32)
    nc.vector.tensor_mul(mask2[:], mask_sb[:], mask_sb[:])
    a_t = prep_pool.tile([IH, K], f32)
    nc.vector.tensor_mul(a_t[:], mask2[:], keys_sb[:])
    ak = prep_pool.tile([IH, K], f32)
    nc.vector.tensor_mul(ak[:], a_t[:], keys_sb[:])
    kn2 = prep_pool.tile([IH, 1], f32)
    nc.vector.reduce_sum(kn2[:], ak[:], axis=mybir.AxisListType.X)
    # softplus(x) = ln(1 + e^x); no Softplus ACT table on this build.
    # strengths ~ N(0,1) so e^x is comfortably in fp32 range.
    es = prep_pool.tile([IH, 1], f32)
    nc.scalar.activation(es[:], str_sb[:], AF.Exp)
    sp = prep_pool.tile([IH, 1], f32)
    nc.scalar.activation(sp[:], es[:], AF.Ln, bias=1.0)

    a_s = prep_pool.tile([IH, K], f32)  # softplus(str) * mask^2 * keys
    nc.vector.tensor_scalar_mul(a_s[:], a_t[:], sp[:])
    b_s = prep_pool.tile([IH, K], f32)  # kn2 * mask^2
    nc.vector.tensor_scalar_mul(b_s[:], mask2[:], kn2[:])

    # transpose [32,128] -> [128,32] on the PE
    prep_ps = tiny_pool.tile([128, 64], f32, tag="tiny")
    nc.tensor.transpose(prep_ps[:, 0:32], a_s[:], identity[0:IH, 0:IH])
    nc.tensor.transpose(prep_ps[:, 32:64], b_s[:], identity[0:IH, 0:IH])

    # zero-padded stationary variants: for (i, o) a [128,32] tile whose
    # columns 8o..8o+8 hold a'_i (resp b'_i); everything else zero.
    lhsA = const_pool.tile([128, B_LOC * 4 * 32], f32)
    lhsB = const_pool.tile([128, B_LOC * 4 * 32], f32)
    nc.vector.memset(lhsA[:], 0.0)
    nc.vector.memset(lhsB[:], 0.0)
    for i in range(B_LOC):
        for o in range(4):
            v = i * 4 + o
            nc.vector.tensor_copy(
                lhsA[:, v * 32 + 8 * o : v * 32 + 8 * o + 8],
                prep_ps[:, 8 * i : 8 * i + 8],
            )
            nc.vector.tensor_copy(
                lhsB[:, v * 32 + 8 * o : v * 32 + 8 * o + 8],
                prep_ps[:, 32 + 8 * i : 32 + 8 * i + 8],
            )

    # onehot helpers for cross-partition fold/broadcast over h = p % 8
    oneT = const_pool.tile([H, 128], f32)  # oneT[h, 8r+h'] = (h==h')
    for r in range(16):
        nc.vector.tensor_copy(oneT[:, 8 * r : 8 * r + 8], identity[0:H, 0:H])
    oh_ps = tiny_pool.tile([128, H], f32, tag="tiny")
    nc.tensor.transpose(oh_ps[:], oneT[:], identity[0:H, 0:H])
    onehot = const_pool.tile([128, H], f32)  # onehot[p, h] = (p%8==h)
    nc.vector.tensor_copy(onehot[:], oh_ps[:])

    # ---- main loop ----------------------------------------------------------
    for m in range(B_LOC * NT):
        i, tp = divmod(m, NT)

        nat = nat_pool.tile([128, MEGA], f32, tag="nat")
        nc.sync.dma_start(
            nat[:].rearrange("p (blk k) -> p blk k", blk=NBLK),
            mem_d[i, tp * MEGA : (tp + 1) * MEGA, :].rearrange(
                "(blk p) k -> p blk k", p=128
            ),
        )

        psumT = psumT_pool.tile([128, MEGA], f32, tag="psumT")
        for b in range(NBLK):
            nc.tensor.transpose(
                psumT[:, b * 128 : (b + 1) * 128],
                nat[:, b * 128 : (b + 1) * 128],
                identity[:],
            )

        memT = memT_pool.tile([128, MEGA], f32, tag="memT")
        nc.vector.tensor_copy(memT[:], psumT[:])
        memT2 = memT2_pool.tile([128, MEGA], f32, tag="memT2")
        nc.scalar.square(memT2[:], psumT[:])

        if tp == 0:
            proj_ps = proj_pool.tile([128, 512], f32, tag="proj")
            msq_ps = msq_pool.tile([128, 512], f32, tag="msq")

        for q in range(NQ):
            t = tp * NQ + q
            g, o = divmod(t, 4)
            v = i * 4 + o
            nc.tensor.matmul(
                proj_ps[32 * g : 32 * g + 32, :],
                lhsA[:, v * 32 : (v + 1) * 32],
                memT[:, q * 512 : (q + 1) * 512],
                start=(o == 0),
                stop=(o == 3),
                tile_position=(0, 32 * g),
            )
            nc.tensor.matmul(
                msq_ps[32 * g : 32 * g + 32, :],
                lhsB[:, v * 32 : (v + 1) * 32],
                memT2[:, q * 512 : (q + 1) * 512],
                start=(o == 0),
                stop=(o == 3),
                tile_position=(0, 32 * g),
            )

        if tp == NT - 1:
            # ---- epilogue for batch i on dense [128,512] tiles -------------
            lnm = epi_pool.tile([128, 512], f32, tag="lnm")
            nc.scalar.activation(lnm[:], msq_ps[:], AF.Ln)
            s_t = epi_pool.tile([128, 512], f32, tag="s_t")
            nc.scalar.activation(s_t[:], lnm[:], AF.Exp, scale=-0.5)
            sharp = epi_pool.tile([128, 512], f32, tag="sharp")
            nc.vector.tensor_mul(sharp[:], proj_ps[:], s_t[:])
            et = epi_pool.tile([128, 512], f32, tag="et")
            sums = small_pool.tile([128, 1], f32, tag="sums")
            nc.scalar.activation(et[:], sharp[:], AF.Exp, accum_out=sums[:])

            # per-h sums across the 16 t-groups: onehot^T @ sums
            hsum_ps = tiny_pool.tile([H, 1], f32, tag="tiny")
            nc.tensor.matmul(
                hsum_ps[:], onehot[:], sums[:], start=True, stop=True
            )
            r8 = small_pool.tile([H, 1], f32, tag="r8")
            nc.vector.reciprocal(r8[:], hsum_ps[:])
            # broadcast back to all 128 partitions: oneT^T @ r8
            rb_ps = tiny_pool.tile([128, 1], f32, tag="tiny")
            nc.tensor.matmul(rb_ps[:], oneT[:], r8[:], start=True, stop=True)
            rb = small_pool.tile([128, 1], f32, tag="rb")
            nc.vector.tensor_copy(rb[:], rb_ps[:])

            out_t = epi_pool.tile([128, 512], f32, tag="out_t")
            nc.vector.tensor_scalar_mul(out_t[:], et[:], rb[:])
            nc.sync.dma_start(
                out_d[i].rearrange("h (t f) -> t h f", t=T_PER_I),
                out_t[:],
            )


def _build():
    from contextlib import ExitStack

    import concourse.bacc as bacc
    import concourse.tile as tile
    from concourse import mybir

    nc = bacc.Bacc(
        "TRN2",
        target_bir_lowering=False,
        debug=False,
        num_devices=N_CORES,
    )
    f32 = mybir.dt.float32
    mem_d = nc.dram_tensor("memory", [B_LOC, J, K], f32, kind="ExternalInput").ap()
    keys_d = nc.dram_tensor("keys", [B_LOC, H, K], f32, kind="ExternalInput").ap()
    str_d = nc.dram_tensor(
        "strengths", [B_LOC, H, 1], f32, kind="ExternalInput"
    ).ap()
    mask_d = nc.dram_tensor("mask", [B_LOC, H, K], f32, kind="ExternalInput").ap()
    out_d = nc.dram_tensor("out", [B_LOC, H, J], f32, kind="ExternalOutput").ap()

    with tile.TileContext(nc) as tc:
        with ExitStack() as ctx:
            _kernel_body(ctx, tc, out_d, mem_d, keys_d, str_d, mask_d)

    nc.compile()
    return nc


def get_nc():
    global _NC
    if _NC is None:
        _NC = _build()
    return _NC


def kernel(memory, keys, strengths, mask):
    global LAST_RESULTS, LAST_EXEC_TIME_NS
    from concourse.bass_utils import run_bass_kernel_spmd

    nc = get_nc()
    in_maps = []
    for c in range(N_CORES):
        sl = slice(c * B_LOC, (c + 1) * B_LOC)
        in_maps.append(
            {
                "memory": np.ascontiguousarray(memory[sl], dtype=np.float32),
                "keys": np.ascontiguousarray(keys[sl], dtype=np.float32),
                "strengths": np.ascontiguousarray(strengths[sl], dtype=np.float32),
                "mask": np.ascontiguousarray(mask[sl], dtype=np.float32),
            }
        )
    res = run_bass_kernel_spmd(nc, in_maps, list(range(N_CORES)))
    LAST_RESULTS = res
    LAST_EXEC_TIME_NS = res.exec_time_ns
    out = np.concatenate([res.results[c]["out"] for c in range(N_CORES)], axis=0)
    return out.astype(np.float32, copy=False)


# revision 7
# speedup vs baseline: 2.0932x; 2.0932x over previous
"""CosineWeights kernel for Trainium2 (Bass/Tile), SPMD over 8 NeuronCores.

Math (per batch i, head h, memory row j):
    mask2   = mask*mask                                  [H,K]
    proj    = sum_k (mask2*keys)[h,k] * mem[j,k]         [H,J]
    msq     = sum_k mask2[h,k] * mem[j,k]^2              [H,J]
    kn2     = sum_k (mask*keys)^2                        [H]
    sharp   = softplus(str)[h] * proj / sqrt(kn2*msq)    (EPS folded away; norm ~40 >> 1e-6)
    out     = softmax_j(sharp)

Sharding: data-parallel over batch dim (32 batches -> 8 cores x 4), no
cross-core communication.

Layout strategy per core:
  - memory arrives [j, k] (k contiguous). PE-transposes 128x128 blocks into
    PSUM [k, j]; DVE copies PSUM->SBUF (memT), ACT squares PSUM->SBUF (memT2).
  - proj/msq matmuls use [K=128, 32] zero-padded stationary tiles (content
    pre-scaled by softplus(str) resp. kn2) placed at column offset 8*(t%4),
    with tile_position col-group g=t//4, accumulating into one dense
    [128, 512] PSUM tile per batch: partition p = 8*t + h, free = j%512.
  - Epilogue per batch runs on the dense [128,512] tiles:
    s = exp(-0.5*ln(msq')) ; sharp = proj'*s ; exp with fused row-sum;
    cross-partition fold/broadcast via tiny onehot matmuls on the PE;
    softmax has no max-subtraction (|sharp| <= ~6 -> exp is safe in fp32).
"""

import os

import numpy as np

B, H, J, K = 32, 8, 8192, 128
N_CORES = 8
B_LOC = B // N_CORES  # 4

MEGA = 1024            # j elements per mega-tile
NBLK = MEGA // 128     # 128x128 transpose blocks per mega-tile
NT = J // MEGA         # mega-tiles per batch
NQ = MEGA // 512       # 512-wide matmul chunks per mega-tile
T_PER_I = J // 512     # 16 (512-)tiles per batch -> packed 8*t+h on 128 partitions

_NC = None
LAST_RESULTS = None
LAST_EXEC_TIME_NS = None


def _kernel_body(ctx, tc, out_d, mem_d, keys_d, str_d, mask_d):
    import concourse.bass as bass
    from concourse import masks, mybir

    nc = tc.nc
    f32 = mybir.dt.float32
    AF = mybir.ActivationFunctionType

    const_pool = ctx.enter_context(tc.tile_pool(name="const", bufs=1))
    prep_pool = ctx.enter_context(tc.tile_pool(name="prep", bufs=1))
    nat_pool = ctx.enter_context(tc.tile_pool(name="nat", bufs=3))
    memT_pool = ctx.enter_context(tc.tile_pool(name="memT", bufs=3))
    memT2_pool = ctx.enter_context(tc.tile_pool(name="memT2", bufs=3))
    epi_pool = ctx.enter_context(tc.tile_pool(name="epi", bufs=2))
    small_pool = ctx.enter_context(tc.tile_pool(name="small", bufs=2))
    psumT_pool = ctx.enter_context(
        tc.tile_pool(name="psumT", bufs=2, space=bass.MemorySpace.PSUM)
    )
    proj_pool = ctx.enter_context(
        tc.tile_pool(name="projps", bufs=1, space=bass.MemorySpace.PSUM)
    )
    msq_pool = ctx.enter_context(
        tc.tile_pool(name="msqps", bufs=1, space=bass.MemorySpace.PSUM)
    )
    tiny_pool = ctx.enter_context(
        tc.tile_pool(name="tinyps", bufs=2, space=bass.MemorySpace.PSUM)
    )

    identity = const_pool.tile([128, 128], f32)
    masks.make_identity(nc, identity[:])
    bf16 = mybir.dt.bfloat16
    identity_bf = const_pool.tile([128, 128], bf16)
    masks.make_identity(nc, identity_bf[:])

    # ---- prep: per-(i,h) scalars and stationary matrices --------------------
    IH = B_LOC * H  # 32

    keys_sb = prep_pool.tile([IH, K], f32)
    nc.sync.dma_start(keys_sb[:], keys_d.rearrange("i h k -> (i h) k"))
    mask_sb = prep_pool.tile([IH, K], f32)
    nc.sync.dma_start(mask_sb[:], mask_d.rearrange("i h k -> (i h) k"))
    str_sb = prep_pool.tile([IH, 1], f32)
    nc.sync.dma_start(str_sb[:], str_d.rearrange("i h one -> (i h) one"))

    mask2 = prep_pool.tile([IH, K], f32)
    nc.vector.tensor_mul(mask2[:], mask_sb[:], mask_sb[:])
    a_t = prep_pool.tile([IH, K], f32)
    nc.vector.tensor_mul(a_t[:], mask2[:], keys_sb[:])
    ak = prep_pool.tile([IH, K], f32)
    nc.vector.tensor_mul(ak[:], a_t[:], keys_sb[:])
    kn2 = prep_pool.tile([IH, 1], f32)
    nc.vector.reduce_sum(kn2[:], ak[:], axis=mybir.AxisListType.X)
    # softplus(x) = ln(1 + e^x); no Softplus ACT table on this build.
    # strengths ~ N(0,1) so e^x is comfortably in fp32 range.
    es = prep_pool.tile([IH, 1], f32)
    nc.scalar.activation(es[:], str_sb[:], AF.Exp)
    sp = prep_pool.tile([IH, 1], f32)
    nc.scalar.activation(sp[:], es[:], AF.Ln, bias=1.0)

    a_s = prep_pool.tile([IH, K], f32)  # softplus(str) * mask^2 * keys
    nc.vector.tensor_scalar_mul(a_s[:], a_t[:], sp[:])
    b_s = prep_pool.tile([IH, K], f32)  # kn2 * mask^2
    nc.vector.tensor_scalar_mul(b_s[:], mask2[:], kn2[:])

    a_sb = prep_pool.tile([IH, K], bf16)
    nc.vector.tensor_copy(a_sb[:], a_s[:])
    b_sb = prep_pool.tile([IH, K], bf16)
    nc.vector.tensor_copy(b_sb[:], b_s[:])

    # transpose [32,128] -> [128,32] on the PE
    prep_ps = tiny_pool.tile([128, 64], bf16, tag="tinybf")
    nc.tensor.transpose(prep_ps[:, 0:32], a_sb[:], identity_bf[0:IH, 0:IH])
    nc.tensor.transpose(prep_ps[:, 32:64], b_sb[:], identity_bf[0:IH, 0:IH])

    # zero-padded stationary variants: for (i, o) a [128,32] tile whose
    # columns 8o..8o+8 hold a'_i (resp b'_i); everything else zero.
    lhsA = const_pool.tile([128, B_LOC * 4 * 32], bf16)
    lhsB = const_pool.tile([128, B_LOC * 4 * 32], bf16)
    nc.vector.memset(lhsA[:], 0.0)
    nc.vector.memset(lhsB[:], 0.0)
    for i in range(B_LOC):
        for o in range(4):
            v = i * 4 + o
            nc.vector.tensor_copy(
                lhsA[:, v * 32 + 8 * o : v * 32 + 8 * o + 8],
                prep_ps[:, 8 * i : 8 * i + 8],
            )
            nc.vector.tensor_copy(
                lhsB[:, v * 32 + 8 * o : v * 32 + 8 * o + 8],
                prep_ps[:, 32 + 8 * i : 32 + 8 * i + 8],
            )

    # onehot helpers for cross-partition fold/broadcast over h = p % 8
    oneT = const_pool.tile([H, 128], f32)  # oneT[h, 8r+h'] = (h==h')
    for r in range(16):
        nc.vector.tensor_copy(oneT[:, 8 * r : 8 * r + 8], identity[0:H, 0:H])
    oh_ps = tiny_pool.tile([128, H], f32, tag="tiny")
    nc.tensor.transpose(oh_ps[:], oneT[:], identity[0:H, 0:H])
    onehot = const_pool.tile([128, H], f32)  # onehot[p, h] = (p%8==h)
    nc.vector.tensor_copy(onehot[:], oh_ps[:])

    # ---- main loop ----------------------------------------------------------
    for m in range(B_LOC * NT):
        i, tp = divmod(m, NT)

        nat = nat_pool.tile([128, MEGA], bf16, tag="nat")
        nc.gpsimd.dma_start(
            nat[:].rearrange("p (blk k) -> p blk k", blk=NBLK),
            mem_d[i, tp * MEGA : (tp + 1) * MEGA, :].rearrange(
                "(blk p) k -> p blk k", p=128
            ),
        )

        psumT = psumT_pool.tile([128, MEGA], bf16, tag="psumT")
        for b in range(NBLK):
            nc.tensor.transpose(
                psumT[:, b * 128 : (b + 1) * 128],
                nat[:, b * 128 : (b + 1) * 128],
                identity_bf[:],
            )

        memT = memT_pool.tile([128, MEGA], bf16, tag="memT")
        nc.vector.tensor_copy(memT[:], psumT[:])
        memT2 = memT2_pool.tile([128, MEGA], bf16, tag="memT2")
        if m % 2 == 0:
            nc.vector.tensor_mul(memT2[:], memT[:], memT[:])
        else:
            nc.scalar.square(memT2[:], memT[:])

        if tp == 0:
            proj_ps = proj_pool.tile([128, 512], f32, tag="proj")
            msq_ps = msq_pool.tile([128, 512], f32, tag="msq")

        for q in range(NQ):
            t = tp * NQ + q
            g, o = divmod(t, 4)
            v = i * 4 + o
            nc.tensor.matmul(
                proj_ps[32 * g : 32 * g + 32, :],
                lhsA[:, v * 32 : (v + 1) * 32],
                memT[:, q * 512 : (q + 1) * 512],
                start=(o == 0),
                stop=(o == 3),
                tile_position=(0, 32 * g),
            )
            nc.tensor.matmul(
                msq_ps[32 * g : 32 * g + 32, :],
                lhsB[:, v * 32 : (v + 1) * 32],
                memT2[:, q * 512 : (q + 1) * 512],
                start=(o == 0),
                stop=(o == 3),
                tile_position=(0, 32 * g),
            )

        if tp == NT - 1:
            # ---- epilogue for batch i on dense [128,512] tiles -------------
            lnm = epi_pool.tile([128, 512], f32, tag="lnm")
            nc.scalar.activation(lnm[:], msq_ps[:], AF.Ln)
            s_t = epi_pool.tile([128, 512], f32, tag="s_t")
            nc.scalar.activation(s_t[:], lnm[:], AF.Exp, scale=-0.5)
            sharp = epi_pool.tile([128, 512], f32, tag="sharp")
            nc.vector.tensor_mul(sharp[:], proj_ps[:], s_t[:])
            et = epi_pool.tile([128, 512], f32, tag="et")
            sums = small_pool.tile([128, 1], f32, tag="sums")
            nc.scalar.activation(et[:], sharp[:], AF.Exp, accum_out=sums[:])

            # per-h sums across the 16 t-groups: onehot^T @ sums
            hsum_ps = tiny_pool.tile([H, 1], f32, tag="tiny")
            nc.tensor.matmul(
                hsum_ps[:], onehot[:], sums[:], start=True, stop=True
            )
            r8 = small_pool.tile([H, 1], f32, tag="r8")
            nc.vector.reciprocal(r8[:], hsum_ps[:])
            # broadcast back to all 128 partitions: oneT^T @ r8
            rb_ps = tiny_pool.tile([128, 1], f32, tag="tiny")
            nc.tensor.matmul(rb_ps[:], oneT[:], r8[:], start=True, stop=True)
            rb = small_pool.tile([128, 1], f32, tag="rb")
            nc.vector.tensor_copy(rb[:], rb_ps[:])

            out_t = epi_pool.tile([128, 512], f32, tag="out_t")
            nc.vector.tensor_scalar_mul(out_t[:], et[:], rb[:])
            nc.sync.dma_start(
                out_d[i].rearrange("h (t f) -> t h f", t=T_PER_I),
                out_t[:],
            )


def _build():
    from contextlib import ExitStack

    import concourse.bacc as bacc
    import concourse.tile as tile
    from concourse import mybir

    nc = bacc.Bacc(
        "TRN2",
        target_bir_lowering=False,
        debug=False,
        num_devices=N_CORES,
    )
    f32 = mybir.dt.float32
    mem_d = nc.dram_tensor("memory", [B_LOC, J, K], f32, kind="ExternalInput").ap()
    keys_d = nc.dram_tensor("keys", [B_LOC, H, K], f32, kind="ExternalInput").ap()
    str_d = nc.dram_tensor(
        "strengths", [B_LOC, H, 1], f32, kind="ExternalInput"
    ).ap()
    mask_d = nc.dram_tensor("mask", [B_LOC, H, K], f32, kind="ExternalInput").ap()
    out_d = nc.dram_tensor("out", [B_LOC, H, J], f32, kind="ExternalOutput").ap()

    with tile.TileContext(nc) as tc:
        with ExitStack() as ctx:
            _kernel_body(ctx, tc, out_d, mem_d, keys_d, str_d, mask_d)

    nc.compile()
    return nc


def get_nc():
    global _NC
    if _NC is None:
        _NC = _build()
    return _NC


def kernel(memory, keys, strengths, mask):
    global LAST_RESULTS, LAST_EXEC_TIME_NS
    from concourse.bass_utils import run_bass_kernel_spmd

    nc = get_nc()
    in_maps = []
    for c in range(N_CORES):
        sl = slice(c * B_LOC, (c + 1) * B_LOC)
        in_maps.append(
            {
                "memory": np.ascontiguousarray(memory[sl], dtype=np.float32),
                "keys": np.ascontiguousarray(keys[sl], dtype=np.float32),
                "strengths": np.ascontiguousarray(strengths[sl], dtype=np.float32),
                "mask": np.ascontiguousarray(mask[sl], dtype=np.float32),
            }
        )
    res = run_bass_kernel_spmd(nc, in_maps, list(range(N_CORES)))
    LAST_RESULTS = res
    LAST_EXEC_TIME_NS = res.exec_time_ns
    out = np.concatenate([res.results[c]["out"] for c in range(N_CORES)], axis=0)
    return out.astype(np.float32, copy=False)


# revision 11
# speedup vs baseline: 2.1507x; 1.0274x over previous
"""CosineWeights kernel for Trainium2 (Bass/Tile), SPMD over 8 NeuronCores.

Math (per batch i, head h, memory row j):
    mask2   = mask*mask                                  [H,K]
    proj    = sum_k (mask2*keys)[h,k] * mem[j,k]         [H,J]
    msq     = sum_k mask2[h,k] * mem[j,k]^2              [H,J]
    kn2     = sum_k (mask*keys)^2                        [H]
    sharp   = softplus(str)[h] * proj / sqrt(kn2*msq)    (EPS folded away; norm ~40 >> 1e-6)
    out     = softmax_j(sharp)

Sharding: data-parallel over batch dim (32 batches -> 8 cores x 4), no
cross-core communication.

Layout strategy per core:
  - memory arrives [j, k] (k contiguous). PE-transposes 128x128 blocks into
    PSUM [k, j]; DVE copies PSUM->SBUF (memT), ACT squares PSUM->SBUF (memT2).
  - proj/msq matmuls use [K=128, 32] zero-padded stationary tiles (content
    pre-scaled by softplus(str) resp. kn2) placed at column offset 8*(t%4),
    with tile_position col-group g=t//4, accumulating into one dense
    [128, 512] PSUM tile per batch: partition p = 8*t + h, free = j%512.
  - Epilogue per batch runs on the dense [128,512] tiles:
    s = exp(-0.5*ln(msq')) ; sharp = proj'*s ; exp with fused row-sum;
    cross-partition fold/broadcast via tiny onehot matmuls on the PE;
    softmax has no max-subtraction (|sharp| <= ~6 -> exp is safe in fp32).
"""

import os

import numpy as np

B, H, J, K = 32, 8, 8192, 128
N_CORES = 8
B_LOC = B // N_CORES  # 4

MEGA = 1024            # j elements per mega-tile
NBLK = MEGA // 128     # 128x128 transpose blocks per mega-tile
NT = J // MEGA         # mega-tiles per batch
NQ = MEGA // 512       # 512-wide matmul chunks per mega-tile
T_PER_I = J // 512     # 16 (512-)tiles per batch -> packed 8*t+h on 128 partitions

_NC = None
LAST_RESULTS = None
LAST_EXEC_TIME_NS = None


def _kernel_body(ctx, tc, out_d, mem_d, keys_d, str_d, mask_d):
    import concourse.bass as bass
    from concourse import masks, mybir

    nc = tc.nc
    f32 = mybir.dt.float32
    AF = mybir.ActivationFunctionType

    const_pool = ctx.enter_context(tc.tile_pool(name="const", bufs=1))
    prep_pool = ctx.enter_context(tc.tile_pool(name="prep", bufs=1))
    nat_pool = ctx.enter_context(tc.tile_pool(name="nat", bufs=3))
    memT_pool = ctx.enter_context(tc.tile_pool(name="memT", bufs=3))
    memT2_pool = ctx.enter_context(tc.tile_pool(name="memT2", bufs=3))
    epi_pool = ctx.enter_context(tc.tile_pool(name="epi", bufs=2))
    small_pool = ctx.enter_context(tc.tile_pool(name="small", bufs=2))
    psumT_pool = ctx.enter_context(
        tc.tile_pool(name="psumT", bufs=2, space=bass.MemorySpace.PSUM)
    )
    proj_pool = ctx.enter_context(
        tc.tile_pool(name="projps", bufs=1, space=bass.MemorySpace.PSUM)
    )
    msq_pool = ctx.enter_context(
        tc.tile_pool(name="msqps", bufs=1, space=bass.MemorySpace.PSUM)
    )
    tiny_pool = ctx.enter_context(
        tc.tile_pool(name="tinyps", bufs=2, space=bass.MemorySpace.PSUM)
    )

    identity = const_pool.tile([128, 128], f32)
    masks.make_identity(nc, identity[:])
    bf16 = mybir.dt.bfloat16
    identity_bf = const_pool.tile([128, 128], bf16)
    masks.make_identity(nc, identity_bf[:])

    # ---- prep: per-(i,h) scalars and stationary matrices --------------------
    IH = B_LOC * H  # 32

    keys_sb = prep_pool.tile([IH, K], f32)
    nc.sync.dma_start(keys_sb[:], keys_d.rearrange("i h k -> (i h) k"))
    mask_sb = prep_pool.tile([IH, K], f32)
    nc.sync.dma_start(mask_sb[:], mask_d.rearrange("i h k -> (i h) k"))
    str_sb = prep_pool.tile([IH, 1], f32)
    nc.sync.dma_start(str_sb[:], str_d.rearrange("i h one -> (i h) one"))

    mask2 = prep_pool.tile([IH, K], f32)
    nc.vector.tensor_mul(mask2[:], mask_sb[:], mask_sb[:])
    a_t = prep_pool.tile([IH, K], f32)
    nc.vector.tensor_mul(a_t[:], mask2[:], keys_sb[:])
    ak = prep_pool.tile([IH, K], f32)
    nc.vector.tensor_mul(ak[:], a_t[:], keys_sb[:])
    kn2 = prep_pool.tile([IH, 1], f32)
    nc.vector.reduce_sum(kn2[:], ak[:], axis=mybir.AxisListType.X)
    # softplus(x) = ln(1 + e^x); no Softplus ACT table on this build.
    # strengths ~ N(0,1) so e^x is comfortably in fp32 range.
    es = prep_pool.tile([IH, 1], f32)
    nc.scalar.activation(es[:], str_sb[:], AF.Exp)
    sp = prep_pool.tile([IH, 1], f32)
    nc.scalar.activation(sp[:], es[:], AF.Ln, bias=1.0)

    a_s = prep_pool.tile([IH, K], f32)  # softplus(str) * mask^2 * keys
    nc.vector.tensor_scalar_mul(a_s[:], a_t[:], sp[:])
    b_s = prep_pool.tile([IH, K], f32)  # kn2 * mask^2
    nc.vector.tensor_scalar_mul(b_s[:], mask2[:], kn2[:])

    a_sb = prep_pool.tile([IH, K], bf16)
    nc.vector.tensor_copy(a_sb[:], a_s[:])
    b_sb = prep_pool.tile([IH, K], bf16)
    nc.vector.tensor_copy(b_sb[:], b_s[:])

    # transpose [32,128] -> [128,32] on the PE
    prep_ps = tiny_pool.tile([128, 64], bf16, tag="tinybf")
    nc.tensor.transpose(prep_ps[:, 0:32], a_sb[:], identity_bf[0:IH, 0:IH])
    nc.tensor.transpose(prep_ps[:, 32:64], b_sb[:], identity_bf[0:IH, 0:IH])

    # zero-padded stationary variants: for (i, o) a [128,32] tile whose
    # columns 8o..8o+8 hold a'_i (resp b'_i); everything else zero.
    lhsA = const_pool.tile([128, B_LOC * 4 * 32], bf16)
    lhsB = const_pool.tile([128, B_LOC * 4 * 32], bf16)
    nc.vector.memset(lhsA[:], 0.0)
    nc.vector.memset(lhsB[:], 0.0)
    for i in range(B_LOC):
        for o in range(4):
            v = i * 4 + o
            nc.vector.tensor_copy(
                lhsA[:, v * 32 + 8 * o : v * 32 + 8 * o + 8],
                prep_ps[:, 8 * i : 8 * i + 8],
            )
            nc.vector.tensor_copy(
                lhsB[:, v * 32 + 8 * o : v * 32 + 8 * o + 8],
                prep_ps[:, 32 + 8 * i : 32 + 8 * i + 8],
            )

    # onehot helpers for cross-partition fold/broadcast over h = p % 8
    oneT = const_pool.tile([H, 128], f32)  # oneT[h, 8r+h'] = (h==h')
    for r in range(16):
        nc.vector.tensor_copy(oneT[:, 8 * r : 8 * r + 8], identity[0:H, 0:H])
    oh_ps = tiny_pool.tile([128, H], f32, tag="tiny")
    nc.tensor.transpose(oh_ps[:], oneT[:], identity[0:H, 0:H])
    onehot = const_pool.tile([128, H], f32)  # onehot[p, h] = (p%8==h)
    nc.vector.tensor_copy(onehot[:], oh_ps[:])

    # ---- main loop ----------------------------------------------------------
    for m in range(B_LOC * NT):
        i, tp = divmod(m, NT)

        # partition p holds the 8 consecutive j-rows 8p..8p+8 -> each
        # partition's data is one contiguous 4KB DRAM run (peak DMA eff).
        nat = nat_pool.tile([128, MEGA], bf16, tag="nat")
        nc.gpsimd.dma_start(
            nat[:].rearrange("p (c k) -> p c k", c=NBLK),
            mem_d[i, tp * MEGA : (tp + 1) * MEGA, :].rearrange(
                "(p c) k -> p c k", p=128
            ),
        )

        psumT = psumT_pool.tile([128, MEGA], bf16, tag="psumT")
        for b in range(NBLK):
            nc.tensor.transpose(
                psumT[:, b * 128 : (b + 1) * 128],
                nat[:, b * 128 : (b + 1) * 128],
                identity_bf[:],
            )

        memT = memT_pool.tile([128, MEGA], bf16, tag="memT")
        nc.vector.tensor_copy(memT[:], psumT[:])
        memT2 = memT2_pool.tile([128, MEGA], bf16, tag="memT2")
        if m % 2 == 0:
            nc.vector.tensor_mul(memT2[:], memT[:], memT[:])
        else:
            nc.scalar.square(memT2[:], memT[:])

        if tp == 0:
            proj_ps = proj_pool.tile([128, 512], f32, tag="proj")
            msq_ps = msq_pool.tile([128, 512], f32, tag="msq")

        memT_v = memT[:].rearrange("kk (b pp) -> kk b pp", b=NBLK)
        memT2_v = memT2[:].rearrange("kk (b pp) -> kk b pp", b=NBLK)
        for q in range(NQ):
            t = tp * NQ + q
            g, o = divmod(t, 4)
            v = i * 4 + o
            # rhs columns for the j-run [512q, 512q+512): psum_T col 128b+pp
            # holds j = 8*pp + b, so take pp in [64q, 64q+64) across all b.
            nc.tensor.matmul(
                proj_ps[32 * g : 32 * g + 32, :],
                lhsA[:, v * 32 : (v + 1) * 32],
                memT_v[:, :, 64 * q : 64 * q + 64],
                start=(o == 0),
                stop=(o == 3),
                tile_position=(0, 32 * g),
            )
            nc.tensor.matmul(
                msq_ps[32 * g : 32 * g + 32, :],
                lhsB[:, v * 32 : (v + 1) * 32],
                memT2_v[:, :, 64 * q : 64 * q + 64],
                start=(o == 0),
                stop=(o == 3),
                tile_position=(0, 32 * g),
            )

        if tp == NT - 1:
            # ---- epilogue for batch i on dense [128,512] tiles -------------
            lnm = epi_pool.tile([128, 512], f32, tag="lnm")
            nc.scalar.activation(lnm[:], msq_ps[:], AF.Ln)
            s_t = epi_pool.tile([128, 512], f32, tag="s_t")
            nc.scalar.activation(s_t[:], lnm[:], AF.Exp, scale=-0.5)
            sharp = epi_pool.tile([128, 512], f32, tag="sharp")
            nc.vector.tensor_mul(sharp[:], proj_ps[:], s_t[:])
            et = epi_pool.tile([128, 512], f32, tag="et")
            sums = small_pool.tile([128, 1], f32, tag="sums")
            nc.scalar.activation(et[:], sharp[:], AF.Exp, accum_out=sums[:])

            # per-h sums across the 16 t-groups: onehot^T @ sums
            hsum_ps = tiny_pool.tile([H, 1], f32, tag="tiny")
            nc.tensor.matmul(
                hsum_ps[:], onehot[:], sums[:], start=True, stop=True
            )
            r8 = small_pool.tile([H, 1], f32, tag="r8")
            nc.vector.reciprocal(r8[:], hsum_ps[:])
            # broadcast back to all 128 partitions: oneT^T @ r8
            rb_ps = tiny_pool.tile([128, 1], f32, tag="tiny")
            nc.tensor.matmul(rb_ps[:], oneT[:], r8[:], start=True, stop=True)
            rb = small_pool.tile([128, 1], f32, tag="rb")
            nc.vector.tensor_copy(rb[:], rb_ps[:])

            # et free index f = 64*b + pp corresponds to j = 8*pp + b within
            # the row's 512-j run; permute while applying the softmax scale.
            out_t = epi_pool.tile([128, 512], f32, tag="out_t")
            nc.vector.tensor_scalar_mul(
                out_t[:].rearrange("r (pp b) -> r b pp", b=NBLK),
                et[:].rearrange("r (b pp) -> r b pp", b=NBLK),
                rb[:],
            )
            nc.sync.dma_start(
                out_d[i].rearrange("h (t f) -> t h f", t=T_PER_I),
                out_t[:],
            )


def _patch_act_tables():
    """The ACT table-load inserter maps each activation to the first set
    containing it; by default Exp lands in exp_and_others and Ln in
    natural_log, forcing a ~1.5us table switch per Ln<->Exp transition
    (2 per batch epilogue). Reorder so the combined
    natural_log_exp_and_others set is found first -- table loads resolve
    by name, so reordering is safe."""
    import concourse.bacc as bacc

    return  # DISABLED for bisection
    if getattr(bacc, "_cosine_act_tables_patched", False):
        return
    orig = bacc.get_activation_tables

    def patched(arch):
        tables = orig(arch)
        key = "natural_log_exp_and_others"
        if key in tables:
            reordered = {key: tables[key]}
            reordered.update({k: v for k, v in tables.items() if k != key})
            return reordered
        return tables

    bacc.get_activation_tables = patched
    bacc._cosine_act_tables_patched = True


def _build():
    from contextlib import ExitStack

    import concourse.bacc as bacc
    import concourse.tile as tile
    from concourse import mybir

    _patch_act_tables()

    nc = bacc.Bacc(
        "TRN2",
        target_bir_lowering=False,
        debug=False,
        num_devices=N_CORES,
    )
    f32 = mybir.dt.float32
    mem_d = nc.dram_tensor("memory", [B_LOC, J, K], f32, kind="ExternalInput").ap()
    keys_d = nc.dram_tensor("keys", [B_LOC, H, K], f32, kind="ExternalInput").ap()
    str_d = nc.dram_tensor(
        "strengths", [B_LOC, H, 1], f32, kind="ExternalInput"
    ).ap()
    mask_d = nc.dram_tensor("mask", [B_LOC, H, K], f32, kind="ExternalInput").ap()
    out_d = nc.dram_tensor("out", [B_LOC, H, J], f32, kind="ExternalOutput").ap()

    with tile.TileContext(nc) as tc:
        with ExitStack() as ctx:
            _kernel_body(ctx, tc, out_d, mem_d, keys_d, str_d, mask_d)

    nc.compile()
    return nc


def get_nc():
    global _NC
    if _NC is None:
        _NC = _build()
    return _NC


def kernel(memory, keys, strengths, mask):
    global LAST_RESULTS, LAST_EXEC_TIME_NS
    from concourse.bass_utils import run_bass_kernel_spmd

    nc = get_nc()
    in_maps = []
    for c in range(N_CORES):
        sl = slice(c * B_LOC, (c + 1) * B_LOC)
        in_maps.append(
            {
                "memory": np.ascontiguousarray(memory[sl], dtype=np.float32),
                "keys": np.ascontiguousarray(keys[sl], dtype=np.float32),
                "strengths": np.ascontiguousarray(strengths[sl], dtype=np.float32),
                "mask": np.ascontiguousarray(mask[sl], dtype=np.float32),
            }
        )
    res = run_bass_kernel_spmd(nc, in_maps, list(range(N_CORES)))
    LAST_RESULTS = res
    LAST_EXEC_TIME_NS = res.exec_time_ns
    out = np.concatenate([res.results[c]["out"] for c in range(N_CORES)], axis=0)
    return out.astype(np.float32, copy=False)


# revision 13
# speedup vs baseline: 2.3820x; 1.1076x over previous
"""CosineWeights kernel for Trainium2 (Bass/Tile), SPMD over 8 NeuronCores.

Math (per batch i, head h, memory row j):
    mask2   = mask*mask                                  [H,K]
    proj    = sum_k (mask2*keys)[h,k] * mem[j,k]         [H,J]
    msq     = sum_k mask2[h,k] * mem[j,k]^2              [H,J]
    kn2     = sum_k (mask*keys)^2                        [H]
    sharp   = softplus(str)[h] * proj / sqrt(kn2*msq)    (EPS folded away; norm ~40 >> 1e-6)
    out     = softmax_j(sharp)

Sharding: data-parallel over batch dim (32 batches -> 8 cores x 4), no
cross-core communication.

Layout strategy per core:
  - memory arrives [j, k] (k contiguous). PE-transposes 128x128 blocks into
    PSUM [k, j]; DVE copies PSUM->SBUF (memT), ACT squares PSUM->SBUF (memT2).
  - proj/msq matmuls use [K=128, 32] zero-padded stationary tiles (content
    pre-scaled by softplus(str) resp. kn2) placed at column offset 8*(t%4),
    with tile_position col-group g=t//4, accumulating into one dense
    [128, 512] PSUM tile per batch: partition p = 8*t + h, free = j%512.
  - Epilogue per batch runs on the dense [128,512] tiles:
    s = exp(-0.5*ln(msq')) ; sharp = proj'*s ; exp with fused row-sum;
    cross-partition fold/broadcast via tiny onehot matmuls on the PE;
    softmax has no max-subtraction (|sharp| <= ~6 -> exp is safe in fp32).
"""

import os

import numpy as np

B, H, J, K = 32, 8, 8192, 128
N_CORES = 8
B_LOC = B // N_CORES  # 4

MEGA = 2048            # j elements per mega-tile
NBLK = MEGA // 128     # 128x128 transpose blocks per mega-tile
NT = J // MEGA         # mega-tiles per batch
NQ = MEGA // 512       # 512-wide matmul chunks per mega-tile
T_PER_I = J // 512     # 16 (512-)tiles per batch -> packed 8*t+h on 128 partitions

_NC = None
LAST_RESULTS = None
LAST_EXEC_TIME_NS = None


def _kernel_body(ctx, tc, out_d, mem_d, keys_d, str_d, mask_d):
    import concourse.bass as bass
    from concourse import masks, mybir

    nc = tc.nc
    f32 = mybir.dt.float32
    AF = mybir.ActivationFunctionType

    const_pool = ctx.enter_context(tc.tile_pool(name="const", bufs=1))
    prep_pool = ctx.enter_context(tc.tile_pool(name="prep", bufs=1))
    nat_pool = ctx.enter_context(tc.tile_pool(name="nat", bufs=3))
    memT_pool = ctx.enter_context(tc.tile_pool(name="memT", bufs=3))
    memT2_pool = ctx.enter_context(tc.tile_pool(name="memT2", bufs=3))
    epi_pool = ctx.enter_context(tc.tile_pool(name="epi", bufs=2))
    small_pool = ctx.enter_context(tc.tile_pool(name="small", bufs=2))
    psumT_pool = ctx.enter_context(
        tc.tile_pool(name="psumT", bufs=2, space=bass.MemorySpace.PSUM)
    )
    proj_pool = ctx.enter_context(
        tc.tile_pool(name="projps", bufs=1, space=bass.MemorySpace.PSUM)
    )
    msq_pool = ctx.enter_context(
        tc.tile_pool(name="msqps", bufs=1, space=bass.MemorySpace.PSUM)
    )
    tiny_pool = ctx.enter_context(
        tc.tile_pool(name="tinyps", bufs=1, space=bass.MemorySpace.PSUM)
    )

    identity = const_pool.tile([128, 128], f32)
    masks.make_identity(nc, identity[:])
    bf16 = mybir.dt.bfloat16
    identity_bf = const_pool.tile([128, 128], bf16)
    masks.make_identity(nc, identity_bf[:])

    # ---- prep: per-(i,h) scalars and stationary matrices --------------------
    IH = B_LOC * H  # 32

    keys_sb = prep_pool.tile([IH, K], f32)
    nc.sync.dma_start(keys_sb[:], keys_d.rearrange("i h k -> (i h) k"))
    mask_sb = prep_pool.tile([IH, K], f32)
    nc.sync.dma_start(mask_sb[:], mask_d.rearrange("i h k -> (i h) k"))
    str_sb = prep_pool.tile([IH, 1], f32)
    nc.sync.dma_start(str_sb[:], str_d.rearrange("i h one -> (i h) one"))

    mask2 = prep_pool.tile([IH, K], f32)
    nc.vector.tensor_mul(mask2[:], mask_sb[:], mask_sb[:])
    a_t = prep_pool.tile([IH, K], f32)
    nc.vector.tensor_mul(a_t[:], mask2[:], keys_sb[:])
    ak = prep_pool.tile([IH, K], f32)
    nc.vector.tensor_mul(ak[:], a_t[:], keys_sb[:])
    kn2 = prep_pool.tile([IH, 1], f32)
    nc.vector.reduce_sum(kn2[:], ak[:], axis=mybir.AxisListType.X)
    # softplus(x) = ln(1 + e^x); no Softplus ACT table on this build.
    # strengths ~ N(0,1) so e^x is comfortably in fp32 range.
    es = prep_pool.tile([IH, 1], f32)
    nc.scalar.activation(es[:], str_sb[:], AF.Exp)
    sp = prep_pool.tile([IH, 1], f32)
    nc.scalar.activation(sp[:], es[:], AF.Ln, bias=1.0)

    a_s = prep_pool.tile([IH, K], f32)  # softplus(str) * mask^2 * keys
    nc.vector.tensor_scalar_mul(a_s[:], a_t[:], sp[:])
    b_s = prep_pool.tile([IH, K], f32)  # kn2 * mask^2
    nc.vector.tensor_scalar_mul(b_s[:], mask2[:], kn2[:])

    a_sb = prep_pool.tile([IH, K], bf16)
    nc.vector.tensor_copy(a_sb[:], a_s[:])
    b_sb = prep_pool.tile([IH, K], bf16)
    nc.vector.tensor_copy(b_sb[:], b_s[:])

    # transpose [32,128] -> [128,32] on the PE
    prep_ps = tiny_pool.tile([128, 64], bf16, tag="tiny")
    nc.tensor.transpose(prep_ps[:, 0:32], a_sb[:], identity_bf[0:IH, 0:IH])
    nc.tensor.transpose(prep_ps[:, 32:64], b_sb[:], identity_bf[0:IH, 0:IH])

    # zero-padded stationary variants: for (i, o) a [128,32] tile whose
    # columns 8o..8o+8 hold a'_i (resp b'_i); everything else zero.
    lhsA = const_pool.tile([128, B_LOC * 4 * 32], bf16)
    lhsB = const_pool.tile([128, B_LOC * 4 * 32], bf16)
    nc.vector.memset(lhsA[:], 0.0)
    nc.vector.memset(lhsB[:], 0.0)
    for i in range(B_LOC):
        for o in range(4):
            v = i * 4 + o
            nc.vector.tensor_copy(
                lhsA[:, v * 32 + 8 * o : v * 32 + 8 * o + 8],
                prep_ps[:, 8 * i : 8 * i + 8],
            )
            nc.vector.tensor_copy(
                lhsB[:, v * 32 + 8 * o : v * 32 + 8 * o + 8],
                prep_ps[:, 32 + 8 * i : 32 + 8 * i + 8],
            )

    # onehot helpers for cross-partition fold/broadcast over h = p % 8
    oneT = const_pool.tile([H, 128], f32)  # oneT[h, 8r+h'] = (h==h')
    for r in range(16):
        nc.vector.tensor_copy(oneT[:, 8 * r : 8 * r + 8], identity[0:H, 0:H])
    oh_ps = tiny_pool.tile([128, H], f32, tag="tiny")
    nc.tensor.transpose(oh_ps[:], oneT[:], identity[0:H, 0:H])
    onehot = const_pool.tile([128, H], f32)  # onehot[p, h] = (p%8==h)
    nc.vector.tensor_copy(onehot[:], oh_ps[:])

    # ---- main loop ----------------------------------------------------------
    for m in range(B_LOC * NT):
        i, tp = divmod(m, NT)

        # partition p holds the 8 consecutive j-rows 8p..8p+8 -> each
        # partition's data is one contiguous 4KB DRAM run (peak DMA eff).
        nat = nat_pool.tile([128, MEGA], bf16, tag="nat")
        nc.gpsimd.dma_start(
            nat[:].rearrange("p (c k) -> p c k", c=NBLK),
            mem_d[i, tp * MEGA : (tp + 1) * MEGA, :].rearrange(
                "(p c) k -> p c k", p=128
            ),
        )

        psumT = psumT_pool.tile([128, MEGA], bf16, tag="psumT")
        for b in range(NBLK):
            nc.tensor.transpose(
                psumT[:, b * 128 : (b + 1) * 128],
                nat[:, b * 128 : (b + 1) * 128],
                identity_bf[:],
            )

        memT = memT_pool.tile([128, MEGA], bf16, tag="memT")
        nc.vector.tensor_copy(memT[:], psumT[:])
        memT2 = memT2_pool.tile([128, MEGA], bf16, tag="memT2")
        if m % 3 == 0:
            nc.vector.tensor_mul(memT2[:], memT[:], memT[:])
        elif m % 3 == 1:
            nc.scalar.square(memT2[:], memT[:])
        else:
            nc.gpsimd.tensor_mul(memT2[:], memT[:], memT[:])

        if tp == 0:
            proj_ps = proj_pool.tile([128, 512], f32, tag="proj")
            msq_ps = msq_pool.tile([128, 512], f32, tag="msq")

        memT_v = memT[:].rearrange("kk (b pp) -> kk b pp", b=NBLK)
        memT2_v = memT2[:].rearrange("kk (b pp) -> kk b pp", b=NBLK)
        for q in range(NQ):
            t = tp * NQ + q
            g, o = divmod(t, 4)
            v = i * 4 + o
            # rhs columns for the j-run [512q, 512q+512): psum_T col 128b+pp
            # holds j = NBLK*pp + b, so take pp in [PQ*q, PQ*q+PQ) across all b.
            PQ = 512 // NBLK
            nc.tensor.matmul(
                proj_ps[32 * g : 32 * g + 32, :],
                lhsA[:, v * 32 : (v + 1) * 32],
                memT_v[:, :, PQ * q : PQ * q + PQ],
                start=(o == 0),
                stop=(o == 3),
                tile_position=(0, 32 * g),
            )
            nc.tensor.matmul(
                msq_ps[32 * g : 32 * g + 32, :],
                lhsB[:, v * 32 : (v + 1) * 32],
                memT2_v[:, :, PQ * q : PQ * q + PQ],
                start=(o == 0),
                stop=(o == 3),
                tile_position=(0, 32 * g),
            )

        if tp == NT - 1:
            # ---- epilogue for batch i on dense [128,512] tiles -------------
            lnm = epi_pool.tile([128, 512], f32, tag="lnm")
            nc.scalar.activation(lnm[:], msq_ps[:], AF.Ln)
            s_t = epi_pool.tile([128, 512], f32, tag="s_t")
            nc.scalar.activation(s_t[:], lnm[:], AF.Exp, scale=-0.5)
            sharp = epi_pool.tile([128, 512], f32, tag="sharp")
            nc.vector.tensor_mul(sharp[:], proj_ps[:], s_t[:])
            et = epi_pool.tile([128, 512], f32, tag="et")
            sums = small_pool.tile([128, 1], f32, tag="sums")
            nc.scalar.activation(et[:], sharp[:], AF.Exp, accum_out=sums[:])

            # per-h sums across the 16 t-groups: onehot^T @ sums
            hsum_ps = tiny_pool.tile([H, 1], f32, tag="tiny")
            nc.tensor.matmul(
                hsum_ps[:], onehot[:], sums[:], start=True, stop=True
            )
            r8 = small_pool.tile([H, 1], f32, tag="r8")
            nc.vector.reciprocal(r8[:], hsum_ps[:])
            # broadcast back to all 128 partitions: oneT^T @ r8
            rb_ps = tiny_pool.tile([128, 1], f32, tag="tiny")
            nc.tensor.matmul(rb_ps[:], oneT[:], r8[:], start=True, stop=True)
            rb = small_pool.tile([128, 1], f32, tag="rb")
            nc.vector.tensor_copy(rb[:], rb_ps[:])

            # et free index f = 64*b + pp corresponds to j = 8*pp + b within
            # the row's 512-j run; permute while applying the softmax scale.
            out_t = epi_pool.tile([128, 512], f32, tag="out_t")
            nc.vector.tensor_scalar_mul(
                out_t[:].rearrange("r (pp b) -> r b pp", b=NBLK),
                et[:].rearrange("r (b pp) -> r b pp", b=NBLK),
                rb[:],
            )
            nc.sync.dma_start(
                out_d[i].rearrange("h (t f) -> t h f", t=T_PER_I),
                out_t[:],
            )


def _patch_act_tables():
    """The ACT table-load inserter maps each activation to the first set
    containing it; by default Exp lands in exp_and_others and Ln in
    natural_log, forcing a ~1.5us table switch per Ln<->Exp transition
    (2 per batch epilogue). Reorder so the combined
    natural_log_exp_and_others set is found first -- table loads resolve
    by name, so reordering is safe."""
    import concourse.bacc as bacc

    return  # DISABLED for bisection
    if getattr(bacc, "_cosine_act_tables_patched", False):
        return
    orig = bacc.get_activation_tables

    def patched(arch):
        tables = orig(arch)
        key = "natural_log_exp_and_others"
        if key in tables:
            reordered = {key: tables[key]}
            reordered.update({k: v for k, v in tables.items() if k != key})
            return reordered
        return tables

    bacc.get_activation_tables = patched
    bacc._cosine_act_tables_patched = True


def _build():
    from contextlib import ExitStack

    import concourse.bacc as bacc
    import concourse.tile as tile
    from concourse import mybir

    _patch_act_tables()

    nc = bacc.Bacc(
        "TRN2",
        target_bir_lowering=False,
        debug=False,
        num_devices=N_CORES,
        num_swdge_queues=2,
    )
    f32 = mybir.dt.float32
    mem_d = nc.dram_tensor("memory", [B_LOC, J, K], f32, kind="ExternalInput").ap()
    keys_d = nc.dram_tensor("keys", [B_LOC, H, K], f32, kind="ExternalInput").ap()
    str_d = nc.dram_tensor(
        "strengths", [B_LOC, H, 1], f32, kind="ExternalInput"
    ).ap()
    mask_d = nc.dram_tensor("mask", [B_LOC, H, K], f32, kind="ExternalInput").ap()
    out_d = nc.dram_tensor("out", [B_LOC, H, J], f32, kind="ExternalOutput").ap()

    with tile.TileContext(nc) as tc:
        with ExitStack() as ctx:
            _kernel_body(ctx, tc, out_d, mem_d, keys_d, str_d, mask_d)

    nc.compile()
    return nc


def get_nc():
    global _NC
    if _NC is None:
        _NC = _build()
    return _NC


def kernel(memory, keys, strengths, mask):
    global LAST_RESULTS, LAST_EXEC_TIME_NS
    from concourse.bass_utils import run_bass_kernel_spmd

    nc = get_nc()
    in_maps = []
    for c in range(N_CORES):
        sl = slice(c * B_LOC, (c + 1) * B_LOC)
        in_maps.append(
            {
                "memory": np.ascontiguousarray(memory[sl], dtype=np.float32),
                "keys": np.ascontiguousarray(keys[sl], dtype=np.float32),
                "strengths": np.ascontiguousarray(strengths[sl], dtype=np.float32),
                "mask": np.ascontiguousarray(mask[sl], dtype=np.float32),
            }
        )
    res = run_bass_kernel_spmd(nc, in_maps, list(range(N_CORES)))
    LAST_RESULTS = res
    LAST_EXEC_TIME_NS = res.exec_time_ns
    out = np.concatenate([res.results[c]["out"] for c in range(N_CORES)], axis=0)
    return out.astype(np.float32, copy=False)
